# revision 2
# baseline (speedup 1.0000x reference)
"""3-layer GAT on 8 trn2 NeuronCores — v3: dst-sharded + loop-based.

Same math as v2 (dst ownership, one AllGather of [f | 1 | el] per layer,
one-hot PE aggregation with fused z, host-precomputed layer-1 softmax), but
the per-window edge phase and the projection run inside tc.For_i hardware
loops with register-offset (ds/ts) access patterns, and per-chunk DVE work is
batched into whole-window 3-D broadcast ops. Every window processes a uniform
NL lo + NH hi chunks (padded), so the loop body is fully static: total static
program size is a few hundred instructions instead of ~14k.

Projection computes f^T = W^T @ x^T (weights stationary -> static ldweights)
and transposes per 128-node tile on the PE; el/er row vectors come from K=1
matmuls against wl/wr, with er rows copied straight into the er_flat row
used by the per-window K=1 broadcast matmul (erB).
"""

import os
import numpy as np

N, E, DIN, DH, DOUT = 50000, 800000, 256, 128, 64
NCORES = 8
PC = 6272             # nodes per core
NPAD = PC * NCORES    # 50176
WPC = 49              # windows (128 dst) per core
P = 128
LOHALF = 32768        # int16 index split


def _wrap16(idx, dtype=np.int16):
    n = len(idx)
    out = np.zeros((P, n // 16), dtype=dtype)
    out[:16, :] = idx.astype(dtype).reshape(-1, 16).T
    out[16:, :] = np.tile(out[:16, :], (7, 1))
    return out


def _leaky(x):
    return np.maximum(x, 0.2 * x)


def host_prep(h, src, dst, W1, al1, ar1, W2, al2, ar2, W3, al3, ar3):
    f32 = np.float32
    h = np.asarray(h, f32)
    src = np.asarray(src, np.int64)
    dst = np.asarray(dst, np.int64)

    hp = np.zeros((NPAD, DIN), f32)
    hp[:N] = h

    wl1 = (np.asarray(W1, f32) @ np.asarray(al1, f32)).astype(f32)
    wr1 = (np.asarray(W1, f32) @ np.asarray(ar1, f32)).astype(f32)
    el1 = hp @ wl1
    er1 = hp @ wr1
    ex1_edge = np.exp(_leaky(el1[src] + er1[dst])).astype(f32)
    z1 = np.zeros(NPAD, f32)
    np.add.at(z1, dst, ex1_edge)
    zrec1 = (1.0 / np.maximum(z1, 1e-9)).astype(f32)

    core_of = dst // PC
    win_of = (dst % PC) >> 7
    cls_of = (src >= LOHALF).astype(np.int64)

    cnt = np.zeros((NCORES, WPC, 2), np.int64)
    np.add.at(cnt, (core_of, win_of, cls_of), 1)
    mx = cnt.max(axis=0)
    NL = int(max(1, -(-mx[:, 0].max() // P)))
    NH = int(max(1, -(-mx[:, 1].max() // P)))
    CW = NL + NH
    TOT = WPC * CW

    in_maps = []
    for c in range(NCORES):
        sel = np.nonzero(core_of == c)[0]
        e_src = src[sel]
        e_dst = dst[sel]
        e_w = win_of[sel]
        e_cls = cls_of[sel]
        e_ex1 = ex1_edge[sel]

        key = e_w * 2 + e_cls
        order = np.argsort(key, kind="stable")
        e_src, e_dst, e_w, e_cls, e_ex1 = (
            e_src[order], e_dst[order], e_w[order], e_cls[order], e_ex1[order])
        ks = key[order]
        pos = np.arange(len(sel), dtype=np.int64)
        run_start = np.zeros(2 * WPC, np.int64)
        uniq, first = np.unique(ks, return_index=True)
        run_start[uniq] = first
        pos -= run_start[ks]

        slot = (e_w * CW + e_cls * NL) * P + pos
        srcadj = np.zeros(TOT * P, np.int64)
        dstloc = np.full(TOT * P, -1.0, f32)
        exv = np.zeros(TOT * P, f32)
        srcadj[slot] = e_src - LOHALF * e_cls
        dstloc[slot] = (e_dst & 127).astype(f32)
        exv[slot] = e_ex1

        m = dict(
            hT=np.ascontiguousarray(hp[c * PC:(c + 1) * PC].T),
            sidx=_wrap16(srcadj),
            dstloc=np.ascontiguousarray(dstloc.reshape(TOT, P).T),
            ex1=np.ascontiguousarray(exv.reshape(TOT, P).T),
            zrec1=np.ascontiguousarray(
                zrec1[c * PC:(c + 1) * PC].reshape(WPC, P).T),
            iota_t=np.tile(np.arange(P, dtype=f32), (P, 1)),
            ident_t=np.eye(P, dtype=f32),
            W1=np.asarray(W1, f32),
            W2=np.asarray(W2, f32),
            W3=np.asarray(W3, f32),
            wl2=(np.asarray(W2, f32) @ np.asarray(al2, f32)).reshape(DH, 1),
            wr2=(np.asarray(W2, f32) @ np.asarray(ar2, f32)).reshape(DH, 1),
            wl3=(np.asarray(W3, f32) @ np.asarray(al3, f32)).reshape(DH, 1),
            wr3=(np.asarray(W3, f32) @ np.asarray(ar3, f32)).reshape(DH, 1),
        )
        in_maps.append(m)

    return dict(in_maps=in_maps, TOT=TOT, NL=NL, NH=NH)


# (table width, dn, el column)
LAY = {
    1: dict(dn=DH, tc=128, elcol=None),
    2: dict(dn=DH, tc=192, elcol=DH + 1),
    3: dict(dn=DOUT, tc=128, elcol=DOUT + 1),
}


def build_program(prep):
    import concourse.bacc as bacc
    import concourse.mybir as mybir
    import concourse.tile as tile
    from concourse import library_config
    from concourse.bass import ds, ts

    f32 = mybir.dt.float32
    i16 = mybir.dt.int16
    AF = mybir.ActivationFunctionType
    OP = mybir.AluOpType
    TOT, NL, NH = prep["TOT"], prep["NL"], prep["NH"]
    CW = NL + NH
    maxl = int(os.environ.get("GAT_MAXL", "3"))

    nc = bacc.Bacc("TRN2", target_bir_lowering=False, debug=False,
                   num_devices=NCORES)

    hT_d = nc.dram_tensor("hT", [DIN, PC], f32, kind="ExternalInput")
    sidx_d = nc.dram_tensor("sidx", [P, TOT * 8], i16, kind="ExternalInput")
    dstloc_d = nc.dram_tensor("dstloc", [P, TOT], f32, kind="ExternalInput")
    ex1_d = nc.dram_tensor("ex1", [P, TOT], f32, kind="ExternalInput")
    zrec1_d = nc.dram_tensor("zrec1", [P, WPC], f32, kind="ExternalInput")
    iota_d = nc.dram_tensor("iota_t", [P, P], f32, kind="ExternalInput")
    ident_d = nc.dram_tensor("ident_t", [P, P], f32, kind="ExternalInput")
    W1_d = nc.dram_tensor("W1", [DIN, DH], f32, kind="ExternalInput")
    W2_d = nc.dram_tensor("W2", [DH, DH], f32, kind="ExternalInput")
    W3_d = nc.dram_tensor("W3", [DH, DOUT], f32, kind="ExternalInput")
    wl2_d = nc.dram_tensor("wl2", [DH, 1], f32, kind="ExternalInput")
    wr2_d = nc.dram_tensor("wr2", [DH, 1], f32, kind="ExternalInput")
    wl3_d = nc.dram_tensor("wl3", [DH, 1], f32, kind="ExternalInput")
    wr3_d = nc.dram_tensor("wr3", [DH, 1], f32, kind="ExternalInput")
    out_d = nc.dram_tensor("out", [PC, DOUT], f32, kind="ExternalOutput")

    with tile.TileContext(nc) as tc:
        with (
            tc.tile_pool(name="sbP", bufs=1) as sbP,
            tc.tile_pool(name="sbS", bufs=1) as sbS,
            tc.tile_pool(name="psA", bufs=2, space="PSUM") as psA,
            tc.tile_pool(name="psB", bufs=2, space="PSUM") as psB,
            tc.tile_pool(name="psF", bufs=2, space="PSUM") as psF,
            tc.tile_pool(name="psC", bufs=2, space="PSUM") as psC,
            tc.tile_pool(name="dram", bufs=1, space="DRAM") as dram,
        ):
            nc.gpsimd.load_library(library_config.mlp)

            iota = sbP.tile([P, P], f32, tag="iota")
            nc.sync.dma_start(iota[:], iota_d[:])
            ident = sbP.tile([P, P], f32, tag="ident")
            nc.sync.dma_start(ident[:], ident_d[:])
            ones1 = sbP.tile([1, P], f32, tag="ones1")
            nc.vector.memset(ones1[:], 1.0)

            hT0 = sbP.tile([P, PC], f32, tag="hT0")
            nc.sync.dma_start(hT0[:], hT_d[0:P, :])
            hT1 = sbP.tile([P, PC], f32, tag="hT1")
            nc.sync.dma_start(hT1[:], hT_d[P:DIN, :])
            sidx = sbP.tile([P, TOT * 8], i16, tag="sidx")
            nc.sync.dma_start(sidx[:], sidx_d[:])
            dstloc = sbP.tile([P, TOT], f32, tag="dstloc")
            nc.sync.dma_start(dstloc[:], dstloc_d[:])
            ex1 = sbP.tile([P, TOT], f32, tag="ex1")
            nc.sync.dma_start(ex1[:], ex1_d[:])
            zrec1 = sbP.tile([P, WPC], f32, tag="zrec1")
            nc.sync.dma_start(zrec1[:], zrec1_d[:])

            Wt = {1: [sbP.tile([P, DH], f32, tag=f"w1_{k}", name=f"w1_{k}")
                      for k in range(2)],
                  2: [sbP.tile([P, DH], f32, tag="w2", name="w2")],
                  3: [sbP.tile([P, DOUT], f32, tag="w3", name="w3")]}
            for k in range(2):
                nc.sync.dma_start(Wt[1][k][:], W1_d[k * P:(k + 1) * P, :])
            nc.sync.dma_start(Wt[2][0][:], W2_d[:])
            nc.sync.dma_start(Wt[3][0][:], W3_d[:])
            wv = {}
            for nm, d in (("wl2", wl2_d), ("wr2", wr2_d),
                          ("wl3", wl3_d), ("wr3", wr3_d)):
                t = sbP.tile([P, 1], f32, tag=nm, name=nm)
                nc.sync.dma_start(t[:], d[:])
                wv[nm] = t

            xT = sbP.tile([P, PC], f32, tag="xT")
            er_flat = sbP.tile([1, PC], f32, tag="er_flat")

            # one stage buffer per layer-width; constant-1 column set once
            stage = {tcw: sbP.tile([P, tcw], f32, tag=f"stage{tcw}",
                                   name=f"stage{tcw}")
                     for tcw in (128, 192)}
            nc.vector.memset(stage[192][:, DH:DH + 1], 1.0)      # l2 ones col

            def bc3(ap2, nch):
                return ap2.rearrange("p (c u) -> p c u", u=1).to_broadcast(
                    [P, nch, P])

            def iota_bc(nch):
                return iota[:].rearrange("p (u d) -> p u d", u=1).to_broadcast(
                    [P, nch, P])

            for rep in range(int(os.environ.get("GAT_REPEAT", "1"))):
              tab_own = {l: dram.tile([PC * LAY[l]["tc"]], f32,
                                      name=f"tab_own{l}_{rep}")
                         for l in (1, 2, 3)}
              tab_full = {l: dram.tile([NPAD * LAY[l]["tc"]], f32,
                                       addr_space="Shared",
                                       name=f"tab_full{l}_{rep}")
                          for l in (1, 2, 3)}
              for l in (1, 2, 3):
                if l > maxl:
                    break
                L = LAY[l]
                dn, tcw, elcol = L["dn"], L["tc"], L["elcol"]
                stg = stage[tcw]

                # ---- projection loop ----
                with tc.For_i(0, WPC, 1) as t:
                    if l == 1:
                        xblk = [hT0[:, ts(t, P)], hT1[:, ts(t, P)]]
                    else:
                        xblk = [xT[:, ts(t, P)]]
                    fT_ps = psB.tile([P, P], f32, space="PSUM", tag="pbig",
                                     name="fT_ps")
                    for k in range(len(xblk)):
                        nc.tensor.matmul(fT_ps[0:dn, :], Wt[l][k][:], xblk[k],
                                         start=(k == 0),
                                         stop=(k == len(xblk) - 1))
                    fT_sb = sbS.tile([P, P], f32, tag="fT_sb", name="fT_sb")
                    nc.vector.tensor_copy(fT_sb[0:dn, :], fT_ps[0:dn, :])
                    st_ps = psB.tile([P, P], f32, space="PSUM", tag="pbig",
                                     name="st_ps")
                    nc.tensor.transpose(st_ps[:, 0:dn], fT_sb[0:dn, :],
                                        ident[0:dn, 0:dn])
                    nc.vector.tensor_copy(stg[:, 0:dn], st_ps[:, 0:dn])
                    if l >= 2:
                        el_ps = psC.tile([1, P], f32, space="PSUM",
                                         tag="pcol", name="el_ps")
                        nc.tensor.matmul(el_ps[:], wv[f"wl{l}"][:], xblk[0],
                                         start=True, stop=True)
                        el_sb = sbS.tile([1, P], f32, tag="el_sb",
                                         name="el_sb")
                        nc.vector.tensor_copy(el_sb[:], el_ps[:])
                        elT_ps = psC.tile([P, 1], f32, space="PSUM",
                                          tag="pcol", name="elT_ps")
                        nc.tensor.transpose(elT_ps[:], el_sb[:], ident[0:1, 0:1])
                        nc.vector.tensor_copy(stg[:, elcol:elcol + 1],
                                              elT_ps[:])
                        if l == 3:
                            nc.vector.memset(stg[:, dn:dn + 1], 1.0)
                        er_ps = psC.tile([1, P], f32, space="PSUM",
                                         tag="pcol", name="er_ps")
                        nc.tensor.matmul(er_ps[:], wv[f"wr{l}"][:], xblk[0],
                                         start=True, stop=True)
                        nc.vector.tensor_copy(er_flat[0:1, ts(t, P)],
                                              er_ps[:])
                    nc.sync.dma_start(
                        tab_own[l][ds(t * (P * tcw), P * tcw)].rearrange(
                            "(p c) -> p c", c=tcw),
                        stg[:])

                nc.gpsimd.collective_compute(
                    "AllGather", mybir.AluOpType.bypass,
                    ins=[tab_own[l][:]],
                    outs=[tab_full[l][:]],
                    replica_groups=[list(range(NCORES))])
                tab_v = tab_full[l][:].rearrange("(r c) -> r c", c=tcw)
                tab_vh = tab_v[LOHALF:, :]

                aggw = dn + 1 if l >= 2 else dn

                # ---- edge + finalize loop ----
                with tc.For_i(0, WPC, 1) as w:
                    G = {}
                    for cls, ncls, base in ((0, NL, 0), (1, NH, NL)):
                        G[cls] = sbS.tile([P, ncls, tcw], f32,
                                          tag=f"G{cls}", name=f"G{cls}")
                        nc.gpsimd.dma_gather(
                            G[cls][:], tab_v if cls == 0 else tab_vh,
                            sidx[:, ds(w * (CW * 8) + base * 8, ncls * 8)],
                            ncls * P, ncls * P, tcw, single_packet=False)
                    if l >= 2:
                        erB_ps = psB.tile([P, P], f32, space="PSUM",
                                          tag="pbig", name="erB_ps")
                        nc.tensor.matmul(erB_ps[:], ones1[:],
                                         er_flat[0:1, ts(w, P)],
                                         start=True, stop=True)
                        erB = sbS.tile([P, P], f32, tag="erB", name="erB")
                        nc.vector.tensor_copy(erB[:], erB_ps[:])

                    oex = {}
                    for cls, ncls, base in ((0, NL, 0), (1, NH, NL)):
                        dsl = dstloc[:, ds(w * CW + base, ncls)]
                        mask = sbS.tile([P, ncls, P], f32, tag=f"mask{cls}",
                                        name=f"mask{cls}")
                        nc.vector.tensor_tensor(
                            out=mask[:], in0=bc3(dsl, ncls),
                            in1=iota_bc(ncls), op=OP.is_equal)
                        scr = sbS.tile([P, ncls, P], f32, tag=f"scr{cls}",
                                       name=f"scr{cls}")
                        if l == 1:
                            exw = ex1[:, ds(w * CW + base, ncls)]
                        else:
                            nc.vector.tensor_tensor(
                                out=scr[:], in0=mask[:],
                                in1=erB[:].rearrange(
                                    "p (u d) -> p u d", u=1).to_broadcast(
                                    [P, ncls, P]),
                                op=OP.mult)
                            ere = sbS.tile([P, ncls], f32, tag=f"ere{cls}",
                                           name=f"ere{cls}")
                            nc.vector.tensor_reduce(
                                out=ere[:], in_=scr[:], op=OP.add,
                                axis=mybir.AxisListType.X)
                            sc = sbS.tile([P, ncls], f32, tag=f"sc{cls}",
                                          name=f"sc{cls}")
                            nc.vector.tensor_tensor(
                                out=sc[:],
                                in0=G[cls][:, :, elcol:elcol + 1].rearrange(
                                    "p c u -> p (c u)"),
                                in1=ere[:], op=OP.add)
                            lr = sbS.tile([P, ncls], f32, tag=f"lr{cls}",
                                          name=f"lr{cls}")
                            nc.vector.scalar_tensor_tensor(
                                out=lr[:], in0=sc[:], scalar=0.2, in1=sc[:],
                                op0=OP.mult, op1=OP.max)
                            exw_t = sbS.tile([P, ncls], f32, tag=f"exw{cls}",
                                             name=f"exw{cls}")
                            nc.scalar.activation(exw_t[:], lr[:], AF.Exp)
                            exw = exw_t[:]
                        oex[cls] = scr
                        nc.vector.tensor_tensor(
                            out=scr[:], in0=mask[:], in1=bc3(exw, ncls),
                            op=OP.mult)

                    agg = psA.tile([P, aggw], f32, space="PSUM", tag="pagg",
                                   name="agg")
                    for cls, ncls in ((0, NL), (1, NH)):
                        for j in range(ncls):
                            nc.tensor.matmul(
                                agg[:], oex[cls][:, j, :],
                                G[cls][:, j, 0:aggw],
                                start=(cls == 0 and j == 0),
                                stop=(cls == 1 and j == NH - 1))

                    # ---- finalize ----
                    if l == 1:
                        zrec = zrec1[:, ds(w, 1)]
                    else:
                        zc = sbS.tile([P, 1], f32, tag="zc", name="zc")
                        nc.vector.tensor_scalar(
                            out=zc[:], in0=agg[:, dn:dn + 1], scalar1=1e-9,
                            scalar2=None, op0=OP.max)
                        zrec_t = sbS.tile([P, 1], f32, tag="zrec",
                                          name="zrec")
                        nc.vector.reciprocal(zrec_t[:], zc[:])
                        zrec = zrec_t[:]
                    if l < 3:
                        xw = sbS.tile([P, dn], f32, tag="xw", name="xw")
                        nc.vector.tensor_scalar(
                            out=xw[:], in0=agg[:, 0:dn], scalar1=zrec,
                            scalar2=0.0, op0=OP.mult, op1=OP.max)
                        xtp = psF.tile([P, P], f32, space="PSUM", tag="pxtp",
                                       name="xtp")
                        nc.tensor.transpose(xtp[:], xw[:], ident[:])
                        nc.vector.tensor_copy(xT[:, ts(w, P)], xtp[:])
                    else:
                        xs = sbS.tile([P, DOUT], f32, tag="ls1", name="xs")
                        nc.vector.tensor_scalar(
                            out=xs[:], in0=agg[:, 0:DOUT], scalar1=zrec,
                            scalar2=None, op0=OP.mult)
                        mxt = sbS.tile([P, 1], f32, tag="mx", name="mxt")
                        nc.vector.tensor_reduce(
                            out=mxt[:], in_=xs[:], op=OP.max,
                            axis=mybir.AxisListType.X)
                        xm = sbS.tile([P, DOUT], f32, tag="xm", name="xm")
                        nc.vector.tensor_scalar(
                            out=xm[:], in0=xs[:], scalar1=mxt[:],
                            scalar2=None, op0=OP.subtract)
                        ee = sbS.tile([P, DOUT], f32, tag="ee", name="ee")
                        nc.scalar.activation(ee[:], xm[:], AF.Exp)
                        se = sbS.tile([P, 1], f32, tag="se", name="se")
                        nc.vector.tensor_reduce(
                            out=se[:], in_=ee[:], op=OP.add,
                            axis=mybir.AxisListType.X)
                        ls = sbS.tile([P, 1], f32, tag="lsum", name="ls")
                        nc.scalar.activation(ls[:], se[:], AF.Ln)
                        fo = sbS.tile([P, DOUT], f32, tag="fout", name="fo")
                        nc.vector.tensor_scalar(
                            out=fo[:], in0=xm[:], scalar1=ls[:],
                            scalar2=None, op0=OP.subtract)
                        nc.sync.dma_start(out_d[ds(w * P, P), :], fo[:])

    nc.compile()
    return nc


def kernel(**inputs):
    from concourse.bass_utils import run_bass_kernel_spmd

    prep = host_prep(**inputs)
    nc = build_program(prep)
    res = run_bass_kernel_spmd(nc, prep["in_maps"], core_ids=list(range(NCORES)))
    out = np.concatenate([res.results[c]["out"] for c in range(NCORES)], axis=0)
    return np.ascontiguousarray(out[:N]).astype(np.float32)
